# revision 5
# baseline (speedup 1.0000x reference)
"""BitLinear fake-quant GEMM on 8 trn2 NeuronCores, data-parallel over batch.

Per core: y[s,o] = round(clip(x/a_scale*127)) @ clip(round(w/w_scale),-1,1)^T
          * (w_scale * a_scale / 127),  a_scale = rowmax|x| + eps.

Quantized activations are integers |v|<=127 and weights are ternary, so a
bf16 matmul with fp32 PSUM accumulation is exact integer arithmetic.

v2: pipeline restructured around the PE floor (512 MMs x 216ns = 110.6us):
- weight ternarization on host (bit-identical f32 round-half-even mirror of
  the previous on-device chain), so the PE head waits only on a 2.1 MB DMA
- work split so no sibling engine exceeds ~60% of PE time:
  GpSimd: x loads + quant mul-add | DVE: abs-max reduce + round-sub |
  Scalar: 127/a + a*ws/127 stats, PSUM epilogue, y stores | Sync: transposes
- paired (2 s-tile) DMAs and stats to halve instruction overheads
- y stored as bf16 (tolerance is 2e-2; bf16 is ~2^-9) and upcast on host
"""

import os
import sys

import numpy as np

sys.path.insert(0, "/opt/trn_rl_repo")

import concourse.bacc as bacc
import concourse.mybir as mybir
import concourse.tile as tile
from concourse.bass_utils import run_bass_kernel_spmd

F32 = mybir.dt.float32
BF16 = mybir.dt.bfloat16
AF = mybir.ActivationFunctionType
ALU = mybir.AluOpType

B = 8      # batches == cores
S = 4096   # rows per core
D = 1024   # in features (contraction)
O = 1024   # out features
P = 128
KB = D // P
NT = S // P          # 32 s-tiles
NP = NT // 2         # 16 s-tile pairs
RND = 12582912.0     # 1.5*2**23: (z+RND)-RND == round-half-even(z) for |z|<2**22
EPS = 1e-8

_CACHE = {}
TRACE_DIR = None


def _build():
    nc = bacc.Bacc("TRN2", target_bir_lowering=False, debug=False)
    x_d = nc.dram_tensor("x", [S, D], F32, kind="ExternalInput")
    w_d = nc.dram_tensor("wqT", [D, O], BF16, kind="ExternalInput")
    wsc_d = nc.dram_tensor("wsc", [P, 1], F32, kind="ExternalInput")
    y_d = nc.dram_tensor("y", [S, O], BF16, kind="ExternalOutput")
    xa, wa, sca, ya = x_d.ap(), w_d.ap(), wsc_d.ap(), y_d.ap()

    xa3 = xa.rearrange("(a p) d -> p a d", p=P)   # [P, NT, D]
    ya3 = ya.rearrange("(a p) o -> p a o", p=P)   # [P, NT, O]
    wa3 = wa.rearrange("(a p) o -> p a o", p=P)   # [P, KB, O]

    with tile.TileContext(nc) as tc:
        with (
            tc.tile_pool(name="wqT", bufs=1) as wqT_p,
            tc.tile_pool(name="xg", bufs=5) as xg_p,
            tc.tile_pool(name="stat", bufs=6) as stat_p,
            tc.tile_pool(name="tq", bufs=3) as tq_p,
            tc.tile_pool(name="aq2", bufs=3) as aq2_p,
            tc.tile_pool(name="aqT", bufs=4) as aqT_p,
            tc.tile_pool(name="ysb", bufs=3) as y_p,
            tc.tile_pool(name="psum", bufs=4, space="PSUM") as ps_p,
        ):
            # prologue: scale constant + host-ternarized weights (bf16, [i, o])
            wscb = wqT_p.tile([P, 1], F32, tag="wscb")
            nc.scalar.dma_start(out=wscb[:], in_=sca[:, :])
            ws127_b = wscb[:, 0:1]
            wqT = wqT_p.tile([P, KB, O], BF16, tag="wqT")
            nc.scalar.dma_start(out=wqT[:], in_=wa3[:, :, :])

            xgs, stats, tqs, aq2s, aqTs, psums = {}, {}, {}, {}, {}, {}

            def load_x(q):       # gpsimd SWDGE queue, one dma per pair
                if not (0 <= q < NP):
                    return
                xg = xg_p.tile([P, 2, D], F32, tag="xg")
                nc.gpsimd.dma_start(out=xg[:], in_=xa3[:, 2 * q:2 * q + 2, :])
                xgs[q] = xg

            def reduce_t(t):     # DVE: a = absmax(x_row) per s-row
                if not (0 <= t < NT):
                    return
                q, j = t // 2, t % 2
                if j == 0:
                    stats[q] = (
                        stat_p.tile([P, 2], F32, tag="st", name=f"st{q}"),
                        stat_p.tile([P, 2], F32, tag="rec", name=f"rec{q}"),
                        stat_p.tile([P, 2], F32, tag="epi", name=f"epi{q}"),
                    )
                st, _, _ = stats[q]
                nc.vector.tensor_reduce(
                    st[:, j:j + 1], xgs[q][:, j, :], mybir.AxisListType.X,
                    ALU.max, apply_absolute_value=True,
                )

            def stats_pair(q):   # DVE: rec127 = 127/a ; Scalar: epi = a*ws/127
                if not (0 <= q < NP):
                    return
                st, rec, epi = stats[q]
                nc.vector.reciprocal(rec[:], st[:])
                nc.vector.tensor_scalar(rec[:], rec[:], 127.0, None, ALU.mult)
                nc.scalar.activation(
                    epi[:], st[:], AF.Copy, bias=0.0, scale=ws127_b
                )

            def quant1(t):       # GpSimd: tq = x*rec127 + RND  (f32)
                if not (0 <= t < NT):
                    return
                q, j = t // 2, t % 2
                _, rec, _ = stats[q]
                tq = tq_p.tile([P, D], F32, tag="tq")
                nc.gpsimd.tensor_scalar(
                    tq[:], xgs[q][:, j, :], rec[:, j:j + 1], RND, ALU.mult, ALU.add
                )
                tqs[t] = tq
                if j == 1:
                    del xgs[q]

            def quant2(t):       # DVE: aq = tq - RND -> bf16 (exact ints)
                if not (0 <= t < NT):
                    return
                q, j = t // 2, t % 2
                if j == 0:
                    aq2s[q] = aq2_p.tile([P, 2, D], BF16, tag="aq2", name=f"aq2_{q}")
                nc.vector.tensor_scalar(
                    aq2s[q][:, j, :], tqs.pop(t)[:], RND, None, ALU.subtract
                )

            def transpose(q):    # Sync HWDGE: [s, i] -> [i, s] for both tiles
                if not (0 <= q < NP):
                    return
                aqT = aqT_p.tile([P, 2 * KB, P], BF16, tag="aqT")
                nc.sync.dma_start_transpose(
                    aqT[:], aq2s.pop(q).rearrange("p a d -> p (a d)")
                )
                aqTs[q] = aqT

            def matmuls(q):      # PE: 32 MMs per pair (2 tiles x 2 o-banks x 8)
                if not (0 <= q < NP):
                    return
                aqT = aqTs.pop(q)
                for j in range(2):
                    yt = ps_p.tile([P, O], F32)
                    for bank in range(2):
                        o0 = bank * 512
                        for b2 in range(KB):
                            nc.tensor.matmul(
                                yt[:, o0:o0 + 512], aqT[:, j * KB + b2, :],
                                wqT[:, b2, o0:o0 + 512],
                                start=(b2 == 0), stop=(b2 == KB - 1),
                            )
                    psums[2 * q + j] = yt

            def epilogue(q):     # Scalar: y = psum * epi -> bf16, then store
                if not (0 <= q < NP):
                    return
                _, _, epi = stats.pop(q)
                ysb = y_p.tile([P, 2, O], BF16, tag="ysb")
                for j in range(2):
                    nc.scalar.activation(
                        ysb[:, j, :], psums.pop(2 * q + j)[:], AF.Copy,
                        bias=0.0, scale=epi[:, j:j + 1],
                    )
                nc.scalar.dma_start(out=ya3[:, 2 * q:2 * q + 2, :], in_=ysb[:])

            # prologue x loads (3 pairs deep)
            for q in range(3):
                load_x(q)
            for t in range(2):
                reduce_t(t)
            stats_pair(0)

            # steady state: one s-tile pair per slot.  Lags keep every
            # engine's in-order stream dependency-free at its head:
            # x 3 pairs ahead, reduce/stats 1 pair ahead, transpose 1 pair
            # ahead of the MMs, epilogue 1 pair behind.
            for q in range(NP + 2):
                load_x(q + 3)
                reduce_t(2 * (q + 1))
                reduce_t(2 * (q + 1) + 1)
                stats_pair(q + 1)
                quant1(2 * q)
                quant2(2 * q)
                quant1(2 * q + 1)
                quant2(2 * q + 1)
                transpose(q)
                matmuls(q - 1)
                epilogue(q - 2)
    nc.compile()
    return nc


def _wq_host(weight):
    # mirror of reference's f32 math: ws = mean|w| + eps in f32; ternary via
    # round-half-even(w * (1/ws)) clipped to [-1, 1] (f32, like the device
    # RND-trick chain this replaces).
    m = np.abs(weight.astype(np.float64)).mean()
    ws = np.float32(np.float32(m) + np.float32(EPS))
    recw = np.float32(1.0 / np.float64(ws))
    u = (weight.astype(np.float32) * recw).astype(np.float32)
    wq = np.clip(np.round(u), -1.0, 1.0).astype(np.float32)
    ws127 = np.float32(np.float64(ws) / 127.0)
    return wq, ws127


def kernel(x, weight):
    import ml_dtypes

    x = np.ascontiguousarray(np.asarray(x), dtype=np.float32)
    weight = np.ascontiguousarray(np.asarray(weight), dtype=np.float32)
    assert x.shape == (B, S, D) and weight.shape == (O, D)
    nc = _CACHE.get("nc")
    if nc is None:
        nc = _CACHE["nc"] = _build()
    wq, ws127 = _wq_host(weight)
    wqT = np.ascontiguousarray(wq.T).astype(ml_dtypes.bfloat16)
    wsc = np.full((P, 1), ws127, dtype=np.float32)
    in_maps = [{"x": x[c], "wqT": wqT, "wsc": wsc} for c in range(B)]
    trace = bool(int(os.environ.get("BITLINEAR_TRACE", "0")))
    res = run_bass_kernel_spmd(
        nc, in_maps, list(range(B)), trace=trace, tmpdir=TRACE_DIR
    )
    _CACHE["last"] = res
    return np.stack(
        [res.results[c]["y"].astype(np.float32) for c in range(B)], axis=0
    )


# revision 7
# speedup vs baseline: 1.1038x; 1.1038x over previous
"""BitLinear fake-quant GEMM on 8 trn2 NeuronCores, data-parallel over batch.

Per core: y[s,o] = round(clip(x/a_scale*127)) @ clip(round(w/w_scale),-1,1)^T
          * (w_scale * a_scale / 127),  a_scale = rowmax|x| + eps.

Quantized activations are integers |v|<=127 and weights are ternary, so a
bf16 matmul with fp32 PSUM accumulation is exact integer arithmetic.

v3: pipelined around the PE floor (512 N=512 matmuls).
- weight ternarization on host (bit-identical f32 round-half-even mirror of
  the v1 on-device chain); device loads a 2 MB bf16 ternary weight
- head: first x tiles loaded singly and the second weight half delayed so
  the pair-0 quant chain owns HBM early; first MM ~23us instead of ~38us
- per-slot emission order puts the current pair's quant ahead of lookahead
  reduces so the static scheduler keeps the transpose feed hot
- engine split keeps every sibling engine under ~60% of PE time:
  GpSimd: x loads + quant mul-add | DVE: abs-max reduce, 127/a, round-sub |
  Scalar: a*ws/127, PSUM epilogue, y stores | Sync: transposes only
- y stored as bf16 (tolerance 2e-2; bf16 is ~2^-9) and upcast on host
"""

import os
import sys

import numpy as np

sys.path.insert(0, "/opt/trn_rl_repo")

import concourse.bacc as bacc
import concourse.mybir as mybir
import concourse.tile as tile
from concourse.bass_utils import run_bass_kernel_spmd

F32 = mybir.dt.float32
BF16 = mybir.dt.bfloat16
AF = mybir.ActivationFunctionType
ALU = mybir.AluOpType

B = 8      # batches == cores
S = 4096   # rows per core
D = 1024   # in features (contraction)
O = 1024   # out features
P = 128
KB = D // P
NT = S // P          # 32 s-tiles
NP = NT // 2         # 16 s-tile pairs
RND = 12582912.0     # 1.5*2**23: (z+RND)-RND == round-half-even(z) for |z|<2**22
EPS = 1e-8

_CACHE = {}
TRACE_DIR = None


def _build():
    nc = bacc.Bacc("TRN2", target_bir_lowering=False, debug=False)
    x_d = nc.dram_tensor("x", [S, D], F32, kind="ExternalInput")
    w_d = nc.dram_tensor("wqT", [D, O], BF16, kind="ExternalInput")
    wsc_d = nc.dram_tensor("wsc", [P, 1], F32, kind="ExternalInput")
    y_d = nc.dram_tensor("y", [S, O], BF16, kind="ExternalOutput")
    xa, wa, sca, ya = x_d.ap(), w_d.ap(), wsc_d.ap(), y_d.ap()

    xa3 = xa.rearrange("(a p) d -> p a d", p=P)   # [P, NT, D]
    ya3 = ya.rearrange("(a p) o -> p a o", p=P)   # [P, NT, O]
    wa3 = wa.rearrange("(a p) o -> p a o", p=P)   # [P, KB, O]

    with tile.TileContext(nc) as tc:
        with (
            tc.tile_pool(name="wqT", bufs=1) as wqT_p,
            tc.tile_pool(name="xg", bufs=7) as xg_p,
            tc.tile_pool(name="stat", bufs=10) as stat_p,
            tc.tile_pool(name="tq", bufs=4) as tq_p,
            tc.tile_pool(name="aq2", bufs=5) as aq2_p,
            tc.tile_pool(name="aqT", bufs=7) as aqT_p,
            tc.tile_pool(name="ysb", bufs=4) as y_p,
            tc.tile_pool(name="psum", bufs=4, space="PSUM") as ps_p,
        ):
            # scale constant + host-ternarized weights (bf16, [i, o]).
            # o-bank 0 is loaded first; bank 1 is issued later (behind the
            # pair-0 stats) so the first x tiles own HBM during the ramp.
            wscb = wqT_p.tile([P, 1], F32, tag="wscb")
            nc.scalar.dma_start(out=wscb[:], in_=sca[:, :])
            ws127_b = wscb[:, 0:1]
            wqT = wqT_p.tile([P, KB, O], BF16, tag="wqT")
            nc.scalar.dma_start(out=wqT[:, :, 0:512], in_=wa3[:, :, 0:512])

            xgs, stats, tqs, aq2s, aqTs, psums = {}, {}, {}, {}, {}, {}

            def load_x(q):       # gpsimd SWDGE queue, one dma per pair
                if not (0 <= q < NP):
                    return
                xg = xg_p.tile([P, 2 * D], F32, tag="xg")
                if q == 0:       # two single-tile dmas: reduce(0) starts sooner
                    nc.gpsimd.dma_start(out=xg[:, 0:D], in_=xa3[:, 0, :])
                    nc.gpsimd.dma_start(out=xg[:, D:2 * D], in_=xa3[:, 1, :])
                else:
                    nc.gpsimd.dma_start(out=xg[:], in_=xa3[:, 2 * q:2 * q + 2, :])
                xgs[q] = xg

            def reduce_t(t):     # DVE: a = absmax(x_row) per s-row
                if not (0 <= t < NT):
                    return
                q, j = t // 2, t % 2
                if j == 0:
                    stats[q] = (
                        stat_p.tile([P, 2], F32, tag="st", name=f"st{q}"),
                        stat_p.tile([P, 2], F32, tag="rec", name=f"rec{q}"),
                        stat_p.tile([P, 2], F32, tag="epi", name=f"epi{q}"),
                    )
                st, _, _ = stats[q]
                nc.vector.tensor_reduce(
                    st[:, j:j + 1], xgs[q][:, j * D:(j + 1) * D],
                    mybir.AxisListType.X, ALU.max, apply_absolute_value=True,
                )

            def stats_pair(q):   # DVE: rec127 = 127/a ; Scalar: epi = a*ws/127
                if not (0 <= q < NP):
                    return
                st, rec, epi = stats[q]
                nc.vector.reciprocal(rec[:], st[:])
                nc.vector.tensor_scalar(rec[:], rec[:], 127.0, None, ALU.mult)
                nc.scalar.activation(
                    epi[:], st[:], AF.Copy, bias=0.0, scale=ws127_b
                )

            def quant1(t):       # GpSimd: tq = x*rec127 + RND  (f32)
                if not (0 <= t < NT):
                    return
                q, j = t // 2, t % 2
                _, rec, _ = stats[q]
                tq = tq_p.tile([P, D], F32, tag="tq")
                nc.gpsimd.tensor_scalar(
                    tq[:], xgs[q][:, j * D:(j + 1) * D], rec[:, j:j + 1], RND,
                    ALU.mult, ALU.add,
                )
                tqs[t] = tq
                if j == 1:
                    del xgs[q]

            def quant2(t):       # DVE: aq = tq - RND -> bf16 (exact ints)
                if not (0 <= t < NT):
                    return
                q, j = t // 2, t % 2
                if j == 0:
                    aq2s[q] = aq2_p.tile(
                        [P, 2 * D], BF16, tag="aq2", name=f"aq2_{q}"
                    )
                nc.vector.tensor_scalar(
                    aq2s[q][:, j * D:(j + 1) * D], tqs.pop(t)[:], RND, None,
                    ALU.subtract,
                )

            def transpose(q):    # Sync HWDGE: [s, i] -> [i, s] for both tiles
                if not (0 <= q < NP):
                    return
                aqT = aqT_p.tile([P, 2 * KB, P], BF16, tag="aqT")
                nc.sync.dma_start_transpose(aqT[:], aq2s.pop(q)[:])
                aqTs[q] = aqT

            def matmuls(q):      # PE: 32 MMs per pair (2 tiles x 2 o-banks x 8)
                if not (0 <= q < NP):
                    return
                aqT = aqTs.pop(q)
                for j in range(2):
                    yt = ps_p.tile([P, O], F32)
                    for bank in range(2):
                        o0 = bank * 512
                        for b2 in range(KB):
                            nc.tensor.matmul(
                                yt[:, o0:o0 + 512], aqT[:, j * KB + b2, :],
                                wqT[:, b2, o0:o0 + 512],
                                start=(b2 == 0), stop=(b2 == KB - 1),
                            )
                    psums[2 * q + j] = yt

            def epilogue(q):     # Scalar: y = psum * epi -> bf16, then store
                if not (0 <= q < NP):
                    return
                _, _, epi = stats.pop(q)
                ysb = y_p.tile([P, 2, O], BF16, tag="ysb")
                for j in range(2):
                    nc.scalar.activation(
                        ysb[:, j, :], psums.pop(2 * q + j)[:], AF.Copy,
                        bias=0.0, scale=epi[:, j:j + 1],
                    )
                nc.scalar.dma_start(out=ya3[:, 2 * q:2 * q + 2, :], in_=ysb[:])

            # prologue: x pairs 0-2 in flight, pair-0 stats started
            for q in range(3):
                load_x(q)
            for t in range(2):
                reduce_t(t)
            stats_pair(0)
            # second weight half on the gpsimd queue behind the first three x
            # pairs, so x0 isn't queued behind 2 MB of weights on HBM (within
            # the all-ready set the scheduler keeps emission order per engine)
            nc.gpsimd.dma_start(out=wqT[:, :, 512:1024], in_=wa3[:, :, 512:1024])

            # steady state: one s-tile pair per slot.  Current pair's quant
            # is emitted (= prioritized) ahead of the lookahead reduces.
            for q in range(NP + 1):
                quant1(2 * q)
                quant2(2 * q)
                quant1(2 * q + 1)
                quant2(2 * q + 1)
                transpose(q)
                reduce_t(2 * (q + 1))
                reduce_t(2 * (q + 1) + 1)
                stats_pair(q + 1)
                load_x(q + 3)
                matmuls(q - 1)
                epilogue(q - 1)
    nc.compile()
    return nc


def _wq_host(weight):
    # mirror of reference's f32 math: ws = mean|w| + eps in f32; ternary via
    # round-half-even(w * (1/ws)) clipped to [-1, 1] (f32, like the device
    # RND-trick chain this replaces).
    m = np.abs(weight.astype(np.float64)).mean()
    ws = np.float32(np.float32(m) + np.float32(EPS))
    recw = np.float32(1.0 / np.float64(ws))
    u = (weight.astype(np.float32) * recw).astype(np.float32)
    wq = np.clip(np.round(u), -1.0, 1.0).astype(np.float32)
    ws127 = np.float32(np.float64(ws) / 127.0)
    return wq, ws127


def kernel(x, weight):
    import ml_dtypes

    x = np.ascontiguousarray(np.asarray(x), dtype=np.float32)
    weight = np.ascontiguousarray(np.asarray(weight), dtype=np.float32)
    assert x.shape == (B, S, D) and weight.shape == (O, D)
    nc = _CACHE.get("nc")
    if nc is None:
        nc = _CACHE["nc"] = _build()
    wq, ws127 = _wq_host(weight)
    wqT = np.ascontiguousarray(wq.T).astype(ml_dtypes.bfloat16)
    wsc = np.full((P, 1), ws127, dtype=np.float32)
    in_maps = [{"x": x[c], "wqT": wqT, "wsc": wsc} for c in range(B)]
    trace = bool(int(os.environ.get("BITLINEAR_TRACE", "0")))
    res = run_bass_kernel_spmd(
        nc, in_maps, list(range(B)), trace=trace, tmpdir=TRACE_DIR
    )
    _CACHE["last"] = res
    return np.stack(
        [res.results[c]["y"].astype(np.float32) for c in range(B)], axis=0
    )
